# revision 42
# baseline (speedup 1.0000x reference)
"""CapsNet dynamic-routing kernel for 8 trn2 NeuronCores (pure data parallel).

Math (per batch element b):
  u[n,:]  = squash(W_pc[n] @ x_groups[b,n] + b_pc[n])          n=7 capsules, dim 8
  u_hat[n,m,:] = u[n,:] @ W[n,m]                               m=12 out caps, dim 16
  b_log = 0
  repeat num_iterations:
     c = softmax_m(b_log); s[m] = sum_n c[n,m] u_hat[n,m]; v = squash(s)
     b_log += u_hat . v
  out[m] = |v[m]|

Implementation notes:
  - squash(s) = s|s|/((1+|s|^2)(|s|+eps)); with eps=1e-8 and |s|=O(1) this is
    s/(1+|s|^2) to ~1e-8 relative.
  - softmax without max subtraction (logits bounded, |b| < ~5).
  - iteration 0 has uniform c=1/12 -> s_0 is a static linear map of u, fused
    into the u_hat matmul as 192 extra columns.
  - u_hat columns live in (k, n, m) order (m fastest) so every per-sample
    broadcast multiply has a dense innermost AP (bf16 DVE 2x mode needs
    innermost stride-1 count>=2 on EVERY operand); k-reduction is a dense
    contiguous halving tree; n-reduction is a blockwise halving tree.
    TensorReduce never gets DVE 2x, so trees beat it everywhere.
  - u_hat and the s1 columns are evicted to SEPARATE tiles so u_hat's
    (c,k,n,m) span is contiguous and passes can use merged [p,(ck),n,m]
    views with ck-granular DVE/GpSimd splits (APs max 3 free dims).
  - routing groups are TILE_R=1024 samples wide (two 512-wide stage-1
    sub-tiles feed 8 chunk matmuls) to halve per-instruction fixed costs;
    the first two groups are 512 wide to shorten the pipeline ramp-in.
  - engine split tuned against the CoreSim TimelineSim model (SPL knobs):
    GpSimd runs Add/Mult at 0.42 efficiency (~1.98 ns/elem) vs DVE-2x
    0.52 ns/elem, so GpSimd takes ~20-25%% of each big pass and DVE keeps
    the rest; exp/squares/evictions run on ScalarE; c/d/blog on DVE.
  - sh = 1/(1+|s|^2) via square + bf16 k-tree, with the +1 fused into the
    final tree add (scalar_tensor_tensor); no separate nsq needed since
    |v|^2 = nsq*sh^2 = sh-sh^2, stored as sh^2-sh and flipped inside the
    batched final Sqrt (scale=-1).

Verified on 8 trn2 cores (axon): rel err ~1.4e-2 vs the fp32 reference
(gate 2e-2).  TimelineSim (CoreSim cost model): 482 us vs 584 us for the
previous version of this kernel (-17%%); axon wall-clock differencing is
too noisy (+-5 ms/launch) to resolve the body directly.
"""

import numpy as np

N_CORES = 8
B_TOTAL = 65536
BP = B_TOTAL // N_CORES          # 8192 samples per core
import os
TILE_F = 512                     # stage-1 free width (batch columns)
TILE_R = int(os.environ.get("K_TILE_R", "1024"))  # routing group width
N_T512 = BP // TILE_F            # stage-1 tiles per core
CHUNK = 128                      # routing chunk (batch on partitions)
N_CHUNK = TILE_F // CHUNK        # 4 chunks per 512-tile
N_CAP, D_IN, D_U = 7, 30, 8      # input capsules
M_CAP, D_V = 12, 16              # output capsules
NJ = N_CAP * D_U                 # 56
NMK = N_CAP * M_CAP * D_V        # 1344
MK = M_CAP * D_V                 # 192
NM = N_CAP * M_CAP               # 84
UHW = NMK + MK                   # 1536 = u_hat cols + s1 cols

ROUT_BF16 = True                 # routing big-pass dtype (bf16 2x vs f32 1x)

# DVE/Pool work splits for the big elementwise passes.  Pool (GpSimd) runs
# Add/Multiply at 0.42 efficiency (1.98 ns/elem regardless of dtype) while
# DVE in bf16-2x mode does 0.52 ns/elem, so Pool's share of 2x-eligible
# passes is ~1/5 and f32 smalls lean Pool.  Tuned against TimelineSim.
SPL = {
    "prod_k": 4,      # prod: global k-split (scaled by NCH), Pool high part
    "tprod_ck": 50,   # tprod: DVE gets ck[0:a*CK/64], Pool the rest
    "ktree_nm": 66,   # ktree lvl1/2: DVE nm[0:a], Pool nm[a:84]
    "ntree_ck": 48,   # ntree lvl1/2: DVE ck[0:a*CK/64], Pool the rest
    "smalls_pool": False,  # c_t/d_t/blog on Pool
    "ktree_l2": True,     # split ktree lvl2 across DVE/Pool
    "ntree_l2": True,     # split ntree lvl2 across DVE/Pool
    "zs_pool": False,     # softmax denominator reduce on Pool
    "hiprio": 0,          # priority boost for latency-critical chain ops
    "sqs_dve": False,     # square s on DVE instead of ScalarE
    "nest": False,        # c-aligned splits: independent DVE/Pool tree chains
}

BUFS = {"xin": 2, "s1": 2, "uhp": 2, "prods": 2, "trees": 1, "smalls": 2}

_prog_cache = {}


def _build(num_iterations: int, repeats: int = 1):
    import concourse.bass as bass
    import concourse.bacc as bacc
    import concourse.tile as tile
    from concourse import mybir

    f32 = mybir.dt.float32
    bf16 = mybir.dt.bfloat16
    dt_r = bf16 if ROUT_BF16 else f32
    rr = 2.0 if ROUT_BF16 else 1.0   # dense-TT rate in routing dtype
    AX = mybir.AxisListType
    OP = mybir.AluOpType
    ACT = mybir.ActivationFunctionType

    nc = bacc.Bacc()

    xT = nc.declare_dram_parameter("xT", [210, BP], f32, isOutput=False)
    w1 = nc.declare_dram_parameter("w1", [210, NJ], f32, isOutput=False)
    w2e = nc.declare_dram_parameter("w2e", [NJ, UHW], dt_r, isOutput=False)
    bpc = nc.declare_dram_parameter("bpc", [NJ, 1], f32, isOutput=False)
    bo = nc.declare_dram_parameter("bo", [NJ, NJ], f32, isOutput=False)
    out = nc.declare_dram_parameter("out", [BP, M_CAP], f32, isOutput=True)

    with tile.TileContext(nc) as tc:
        with (
            nc.allow_low_precision(reason="bf16 big passes; accumulations "
                                          "that matter are kept fp32"),
            tc.tile_pool(name="singles", bufs=1) as singles,
            tc.tile_pool(name="xin", bufs=BUFS["xin"]) as xin,
            tc.tile_pool(name="s1pool", bufs=BUFS["s1"]) as s1pool,
            tc.tile_pool(name="uhp", bufs=BUFS["uhp"]) as uhp,
            tc.tile_pool(name="prods", bufs=BUFS["prods"]) as prods,
            tc.tile_pool(name="trees", bufs=BUFS["trees"]) as trees,
            tc.tile_pool(name="smalls", bufs=BUFS["smalls"]) as smalls,
            tc.tile_pool(name="psz", bufs=1, space="PSUM") as psz,
            tc.tile_pool(name="psn", bufs=1, space="PSUM") as psn,
            tc.tile_pool(name="psuh", bufs=2, space="PSUM") as psuh,
        ):
            # ---- load constants once ----
            w1a_s = singles.tile([128, NJ], f32)
            w1b_s = singles.tile([82, NJ], f32)
            w2e_s = singles.tile([NJ, UHW], dt_r)
            bpc_s = singles.tile([NJ, 1], f32)
            bo_s = singles.tile([NJ, NJ], f32)
            qbuf = singles.tile([CHUNK, BP // CHUNK, M_CAP], f32)
            nc.sync.dma_start(out=w1a_s, in_=w1[0:128, :])
            nc.sync.dma_start(out=w1b_s, in_=w1[128:210, :])
            nc.sync.dma_start(out=w2e_s, in_=w2e[:, :])
            nc.sync.dma_start(out=bpc_s, in_=bpc[:, :])
            nc.sync.dma_start(out=bo_s, in_=bo[:, :])

            # narrow ramp-in/ramp-out tiles shorten the pipeline fill/drain
            if TILE_R > TILE_F:
                plan = [TILE_F, TILE_F] + \
                    [TILE_R] * ((BP - 2 * TILE_F) // TILE_R)
            else:
                plan = [TILE_R] * (BP // TILE_R)
            assert sum(plan) == BP
            offs = [sum(plan[:i]) for i in range(len(plan))]
            for t in range(len(plan) * repeats):
                t = t % len(plan)
                W = plan[t]
                NCH = W // CHUNK
                CK = NCH * D_V
                SUB = W // TILE_F
                goff = offs[t] // CHUNK
                # ---- stage 1: primary capsules (feature-major, PE),
                #      at TILE_F granularity; SUB sub-tiles per group ----
                uTbs = []
                for sb in range(SUB):
                    c0 = offs[t] + sb * TILE_F
                    xa = xin.tile([128, TILE_F], f32, tag="xa")
                    xb = xin.tile([82, TILE_F], f32, tag="xb")
                    nc.sync.dma_start(out=xa, in_=xT[0:128, c0:c0 + TILE_F])
                    nc.sync.dma_start(out=xb, in_=xT[128:210, c0:c0 + TILE_F])

                    z = psz.tile([NJ, TILE_F], f32)
                    nc.tensor.matmul(z, w1a_s, xa, start=True, stop=False)
                    nc.tensor.matmul(z, w1b_s, xb, start=False, stop=True)

                    # sq = (z + b_pc)^2   (ACT, bias per partition)
                    sq = s1pool.tile([NJ, TILE_F], f32, tag="sq")
                    nc.scalar.activation(out=sq, in_=z, func=ACT.Square,
                                         bias=bpc_s, scale=1.0)
                    # per-capsule |u_raw|^2, replicated across its 8 rows
                    nsqz = psn.tile([NJ, TILE_F], f32)
                    nc.tensor.matmul(nsqz, bo_s, sq, start=True, stop=True)
                    # f = 1/(1+nsq)
                    pf = s1pool.tile([NJ, TILE_F], f32, tag="pf")
                    nc.scalar.add(pf, nsqz, 1.0)
                    fz = s1pool.tile([NJ, TILE_F], f32, tag="fz")
                    nc.vector.reciprocal(fz, pf)
                    # uT = (z + b_pc) * f
                    uTb = s1pool.tile([NJ, TILE_F], dt_r, tag="uT")
                    nc.vector.scalar_tensor_tensor(
                        out=uTb, in0=z, scalar=bpc_s, in1=fz,
                        op0=OP.add, op1=OP.mult)
                    uTbs.append(uTb)

                # ---- routing group (NCH chunks wide) ----
                # u_hat and the s1 columns live in SEPARATE tiles so the
                # (c,k,n,m) span of u_hat is fully contiguous: passes can
                # use merged [p,(ck),n,m] views with ck-granular DVE/Pool
                # splits.  ISA allows at most 3 free dims per AP.
                uhs = uhp.tile([CHUNK, NCH * NMK], dt_r, tag="uhs")
                s1b = uhp.tile([CHUNK, NCH * MK], dt_r, tag="s1b")
                uhsv = uhs.rearrange("p (c w) -> p c w", c=NCH)
                s1bv = s1b.rearrange("p (c w) -> p c w", c=NCH)
                for cc in range(NCH):
                    uh = psuh.tile([CHUNK, UHW], f32)
                    ut = uTbs[cc * CHUNK // TILE_F]
                    o0 = (cc * CHUNK) % TILE_F
                    lhsT = ut[:, o0:o0 + CHUNK]
                    for j in range(3):
                        nc.tensor.matmul(uh[:, j * 512:(j + 1) * 512], lhsT,
                                         w2e_s[:, j * 512:(j + 1) * 512],
                                         start=True, stop=True)
                    nc.scalar.copy(uhsv[:, cc, :], uh[:, 0:NMK])
                    nc.scalar.copy(s1bv[:, cc, :], uh[:, NMK:UHW])

                def uhat_ckv():
                    # [p, c, k, nm] view of u_hat
                    return uhs.rearrange("p (c k nm) -> p c k nm",
                                         c=NCH, k=D_V)

                def uhat_ck():
                    # merged [p, (ck), n, m] view of u_hat
                    return uhs.rearrange("p (ck n m) -> p ck n m",
                                         ck=CK, n=N_CAP)

                def ktree(src):
                    # sum over k: contiguous halving; views [p, c, k', nm];
                    # engine split on the dense innermost nm axis, or (nest
                    # mode) on the c axis so the Pool chain is independent
                    def kv(ap, kk):
                        return ap.rearrange("p (c k nm) -> p c k nm",
                                            c=NCH, k=kk)
                    a = SPL["ktree_nm"]
                    cn = (SPL["tprod_ck"] * CK // 64) // D_V

                    def lvl(dst, x, y, split):
                        if SPL["nest"]:
                            nc.vector.tensor_add(dst[:, 0:cn], x[:, 0:cn],
                                                 y[:, 0:cn])
                            nc.gpsimd.tensor_add(dst[:, cn:NCH], x[:, cn:NCH],
                                                 y[:, cn:NCH])
                        elif split:
                            nc.vector.tensor_add(dst[:, :, :, 0:a],
                                                 x[:, :, :, 0:a],
                                                 y[:, :, :, 0:a])
                            nc.gpsimd.tensor_add(dst[:, :, :, a:NM],
                                                 x[:, :, :, a:NM],
                                                 y[:, :, :, a:NM])
                        else:
                            nc.vector.tensor_add(dst, x, y)
                    w1_ = trees.tile([CHUNK, NCH * 8 * NM], dt_r, tag="kt1")
                    lvl(kv(w1_, 8), kv(src, 16)[:, :, 0:8, :],
                        kv(src, 16)[:, :, 8:16, :], True)
                    w2_ = trees.tile([CHUNK, NCH * 4 * NM], dt_r, tag="kt2")
                    lvl(kv(w2_, 4), kv(w1_, 8)[:, :, 0:4, :],
                        kv(w1_, 8)[:, :, 4:8, :], SPL["ktree_l2"])
                    w3_ = trees.tile([CHUNK, NCH * 2 * NM], dt_r, tag="kt3")
                    lvl(kv(w3_, 2), kv(w2_, 4)[:, :, 0:2, :],
                        kv(w2_, 4)[:, :, 2:4, :], False)
                    tt = smalls.tile([CHUNK, NCH * NM], f32, tag="t_t")
                    if SPL["nest"]:
                        lvl(kv(tt, 1), kv(w3_, 2)[:, :, 0:1, :],
                            kv(w3_, 2)[:, :, 1:2, :], True)
                    else:
                        nc.vector.tensor_add(kv(tt, 1),
                                             kv(w3_, 2)[:, :, 0:1, :],
                                             kv(w3_, 2)[:, :, 1:2, :])
                    return tt

                def ntree(pc, out_f32=False):
                    # sum over n: blockwise halving; views [p, (ck), n', m]
                    def v(ap, nn):
                        return ap.rearrange("p (ck n m) -> p ck n m",
                                            ck=CK, n=nn)
                    a = SPL["ntree_ck"] * CK // 64

                    def lvl(dst, x, y, split):
                        if split:
                            nc.vector.tensor_add(dst[:, 0:a], x[:, 0:a],
                                                 y[:, 0:a])
                            nc.gpsimd.tensor_add(dst[:, a:CK], x[:, a:CK],
                                                 y[:, a:CK])
                        else:
                            nc.vector.tensor_add(dst, x, y)
                    pcv = v(pc, N_CAP)
                    w1_ = trees.tile([CHUNK, CK * 3 * M_CAP], dt_r,
                                     tag="nt1")
                    lvl(v(w1_, 3), pcv[:, :, 0:3, :], pcv[:, :, 3:6, :],
                        True)
                    y = trees.tile([CHUNK, CK * M_CAP], dt_r, tag="nt2")
                    lvl(v(y, 1), v(w1_, 3)[:, :, 0:1, :],
                        v(w1_, 3)[:, :, 1:2, :], SPL["ntree_l2"])
                    y2 = trees.tile([CHUNK, CK * M_CAP], dt_r, tag="nt3")
                    lvl(v(y2, 1), v(y, 1), v(w1_, 3)[:, :, 2:3, :], False)
                    st = smalls.tile([CHUNK, CK * M_CAP],
                                     f32 if out_f32 else dt_r, tag="s_t")
                    nc.vector.tensor_add(v(st, 1), v(y2, 1),
                                         pcv[:, :, 6:7, :])
                    return st

                import contextlib

                def prio():
                    if SPL["hiprio"]:
                        return tc.high_priority(offset=SPL["hiprio"])
                    return contextlib.nullcontext()

                def sh_of(s_tile):
                    # sh = 1/(1+|s|^2) per (chunk, m): ScalarE square then a
                    # dense bf16 halving tree over k (TensorReduce never gets
                    # 2x); the +1 rides the final tree add via stt.
                    def v4(ap, kk):
                        return ap.rearrange("p (c k m) -> p c k m",
                                            c=NCH, k=kk)
                    ctx_ = prio(); ctx_.__enter__()
                    sqs = smalls.tile([CHUNK, NCH * MK], dt_r, tag="sqs")
                    if SPL["sqs_dve"]:
                        nc.vector.tensor_mul(sqs, s_tile, s_tile)
                    else:
                        nc.scalar.activation(out=sqs, in_=s_tile,
                                             func=ACT.Square)
                    t8 = smalls.tile([CHUNK, NCH * 8 * M_CAP], dt_r,
                                     tag="sq8")
                    nc.vector.tensor_add(v4(t8, 8), v4(sqs, 16)[:, :, 0:8, :],
                                         v4(sqs, 16)[:, :, 8:16, :])
                    t4 = smalls.tile([CHUNK, NCH * 4 * M_CAP], dt_r,
                                     tag="sq4")
                    nc.vector.tensor_add(v4(t4, 4), v4(t8, 8)[:, :, 0:4, :],
                                         v4(t8, 8)[:, :, 4:8, :])
                    t2 = smalls.tile([CHUNK, NCH * 2 * M_CAP], dt_r,
                                     tag="sq2")
                    nc.vector.tensor_add(v4(t2, 2), v4(t4, 4)[:, :, 0:2, :],
                                         v4(t4, 4)[:, :, 2:4, :])
                    p1 = smalls.tile([CHUNK, NCH * M_CAP], f32, tag="p1")
                    nc.vector.scalar_tensor_tensor(
                        out=v4(p1, 1), in0=v4(t2, 2)[:, :, 0:1, :],
                        scalar=1.0, in1=v4(t2, 2)[:, :, 1:2, :],
                        op0=OP.add, op1=OP.add)
                    sh = smalls.tile([CHUNK, NCH * M_CAP], f32, tag="sh")
                    nc.vector.reciprocal(sh, p1)
                    ctx_.__exit__(None, None, None)
                    return sh

                def small_tt(out, in0, in1, op):
                    eng = nc.gpsimd if SPL["smalls_pool"] else nc.vector
                    eng.tensor_tensor(out=out, in0=in0, in1=in1, op=op)

                # ---- iteration 0 (uniform c; s1 precomputed by PE) ----
                s_t = s1b
                sh = sh_of(s_t)

                blog = None
                for it in range(num_iterations):
                    last = (it == num_iterations - 1)
                    if it > 0:
                        # c = softmax_m(blog), [p, (c n m)]
                        ctx_ = prio(); ctx_.__enter__()
                        e = smalls.tile([CHUNK, NCH * NM], dt_r, tag="e")
                        nc.scalar.activation(out=e, in_=blog, func=ACT.Exp)
                        zs = smalls.tile([CHUNK, NCH * N_CAP], f32, tag="zs")
                        zeng = nc.gpsimd if SPL["zs_pool"] else nc.vector
                        zeng.tensor_reduce(
                            zs, e.rearrange("p (c n m) -> p c n m",
                                            c=NCH, n=N_CAP),
                            axis=AX.X, op=OP.add)
                        rz = smalls.tile([CHUNK, NCH * N_CAP], dt_r, tag="rz")
                        nc.vector.reciprocal(rz, zs)
                        c_t = smalls.tile([CHUNK, NCH * NM], dt_r, tag="c_t")
                        small_tt(
                            out=c_t.rearrange("p (c n m) -> p c n m",
                                              c=NCH, n=N_CAP),
                            in0=e.rearrange("p (c n m) -> p c n m",
                                            c=NCH, n=N_CAP),
                            in1=rz.rearrange("p (c n) -> p c n", c=NCH)
                                 .unsqueeze(3)
                                 .broadcast_to([CHUNK, NCH, N_CAP, M_CAP]),
                            op=OP.mult)
                        ctx_.__exit__(None, None, None)
                        # s = sum_n c * u_hat   (bcast over k: [c, k, nm])
                        # global k-split at K of NCH*16: DVE low, Pool high
                        K = NCH * D_V - (D_V - SPL["prod_k"]) * NCH // 4
                        ca, kp = K // D_V, K % D_V
                        pc = prods.tile([CHUNK, NCH * NMK], dt_r, tag="prod")
                        pcv = pc.rearrange("p (c k nm) -> p c k nm",
                                           c=NCH, k=D_V)
                        uv = uhat_ckv()
                        cbc = (c_t.rearrange("p (c nm) -> p c nm", c=NCH)
                               .unsqueeze(2)
                               .broadcast_to([CHUNK, NCH, D_V, NM]))
                        nc.vector.tensor_tensor(
                            out=pcv[:, 0:ca], in0=uv[:, 0:ca],
                            in1=cbc[:, 0:ca], op=OP.mult)
                        if kp:
                            nc.vector.tensor_tensor(
                                out=pcv[:, ca:ca + 1, 0:kp],
                                in0=uv[:, ca:ca + 1, 0:kp],
                                in1=cbc[:, ca:ca + 1, 0:kp], op=OP.mult)
                            nc.gpsimd.tensor_tensor(
                                out=pcv[:, ca:ca + 1, kp:D_V],
                                in0=uv[:, ca:ca + 1, kp:D_V],
                                in1=cbc[:, ca:ca + 1, kp:D_V], op=OP.mult)
                        if ca + 1 < NCH or (not kp and ca < NCH):
                            lo = ca + 1 if kp else ca
                            if lo < NCH:
                                nc.gpsimd.tensor_tensor(
                                    out=pcv[:, lo:NCH], in0=uv[:, lo:NCH],
                                    in1=cbc[:, lo:NCH], op=OP.mult)
                        s_t = ntree(pc, out_f32=last)
                        sh = sh_of(s_t)

                    if not last:
                        # t = sum_k u_hat*s ; merged [p,(ck),n,m] views
                        a = SPL["tprod_ck"] * CK // 64
                        if SPL["nest"]:
                            a = (a // D_V) * D_V
                        pt = prods.tile([CHUNK, NCH * NMK], dt_r, tag="prod")
                        ptv = pt.rearrange("p (ck n m) -> p ck n m",
                                           ck=CK, n=N_CAP)
                        uckv = uhat_ck()
                        sbc = (s_t.rearrange("p (ck m) -> p ck m", ck=CK)
                               .unsqueeze(2)
                               .broadcast_to([CHUNK, CK, N_CAP, M_CAP]))
                        nc.vector.tensor_tensor(
                            out=ptv[:, 0:a], in0=uckv[:, 0:a],
                            in1=sbc[:, 0:a], op=OP.mult)
                        nc.gpsimd.tensor_tensor(
                            out=ptv[:, a:CK], in0=uckv[:, a:CK],
                            in1=sbc[:, a:CK], op=OP.mult)
                        t_t = ktree(pt)
                        ctx_ = prio(); ctx_.__enter__()
                        d_t = smalls.tile([CHUNK, NCH * NM], f32, tag="d_t")
                        small_tt(
                            out=d_t.rearrange("p (c n m) -> p c n m",
                                              c=NCH, n=N_CAP),
                            in0=t_t.rearrange("p (c n m) -> p c n m",
                                              c=NCH, n=N_CAP),
                            in1=sh.rearrange("p (c m) -> p c m", c=NCH)
                                 .unsqueeze(2)
                                 .broadcast_to([CHUNK, NCH, N_CAP, M_CAP]),
                            op=OP.mult)
                        if it == 0:
                            blog = d_t
                        else:
                            nblog = smalls.tile([CHUNK, NCH * NM], f32,
                                                tag="blog")
                            eng = (nc.gpsimd if SPL["smalls_pool"]
                                   else nc.vector)
                            eng.tensor_add(nblog, blog, d_t)
                            blog = nblog
                        ctx_.__exit__(None, None, None)
                    else:
                        # |v|^2 = nsq*sh^2 = sh - sh^2; store sh^2 - sh and
                        # flip sign inside the batched Sqrt (scale=-1)
                        nc.vector.scalar_tensor_tensor(
                            out=qbuf[:, goff:goff + NCH, :]
                            .rearrange("p c m -> p (c m)"),
                            in0=sh, scalar=1.0, in1=sh,
                            op0=OP.subtract, op1=OP.mult)

            # ---- batched final sqrt (in place) + single output DMA ----
            nc.scalar.activation(out=qbuf, in_=qbuf, func=ACT.Sqrt,
                                 scale=-1.0)
            nc.sync.dma_start(
                out=out.rearrange("(g p) m -> p g m", p=CHUNK, g=BP // CHUNK),
                in_=qbuf)
    nc.compile()
    return nc


def _prep_weights(W_pc, b_pc, W):
    W1 = np.zeros((210, NJ), np.float32)
    W2E = np.zeros((NJ, UHW), np.float32)
    BO = np.zeros((NJ, NJ), np.float32)
    for n in range(N_CAP):
        W1[n * D_IN:(n + 1) * D_IN, n * D_U:(n + 1) * D_U] = W_pc[n].T
        BO[n * D_U:(n + 1) * D_U, n * D_U:(n + 1) * D_U] = 1.0
    for n in range(N_CAP):
        for m in range(M_CAP):
            for k in range(D_V):
                # u_hat columns in (k, n, m) order; s1 columns in (k, m)
                W2E[n * D_U:(n + 1) * D_U, k * NM + n * M_CAP + m] = W[n, m, :, k]
                W2E[n * D_U:(n + 1) * D_U, NMK + k * M_CAP + m] += (
                    W[n, m, :, k] / float(M_CAP))
    BPC = b_pc.reshape(NJ, 1).astype(np.float32)
    return W1, W2E, BO, BPC


def _make_in_maps(x, W_pc, b_pc, W):
    W1, W2E, BO, BPC = _prep_weights(W_pc, b_pc, W)
    if ROUT_BF16:
        import ml_dtypes
        W2E = W2E.astype(ml_dtypes.bfloat16)
    xt = np.ascontiguousarray(x.T)                      # [210, B]
    in_maps = []
    for i in range(N_CORES):
        in_maps.append({
            "xT": np.ascontiguousarray(xt[:, i * BP:(i + 1) * BP]),
            "w1": W1, "w2e": W2E, "bpc": BPC, "bo": BO,
        })
    return in_maps


def kernel(x, W_pc, b_pc, W, num_iterations, _trace=False):
    from concourse.bass_utils import run_bass_kernel_spmd

    x = np.asarray(x, np.float32)
    W_pc = np.asarray(W_pc, np.float32)
    b_pc = np.asarray(b_pc, np.float32)
    W = np.asarray(W, np.float32)
    nit = int(num_iterations)
    assert x.shape == (B_TOTAL, 210)

    key = nit
    if key not in _prog_cache:
        _prog_cache[key] = _build(nit)
    nc = _prog_cache[key]

    in_maps = _make_in_maps(x, W_pc, b_pc, W)
    res = run_bass_kernel_spmd(nc, in_maps, list(range(N_CORES)),
                               trace=_trace)
    outs = [res.results[i]["out"] for i in range(N_CORES)]
    full = np.concatenate(outs, axis=0)
    if _trace:
        kernel._last_exec_time_ns = res.exec_time_ns
        kernel._last_results = res
    return full

